# Initial kernel scaffold
#
"""Trainium2 Bass kernel for fused attention + LayerNorm + projection.

Computation (per reference):
    q = input1 @ Wq + bq                       [8192, 32]
    k = input2 @ Wk + bk                       [8192, 32]
    v = input2 @ Wv + bv                       [8192, 32]
    P = softmax(q @ k.T, axis=-1)              [8192, 8192]
    fused = P @ v                              [8192, 32]
    out = LayerNorm(fused) * gamma + beta @ Wo + bo   [8192, 128]

Sharding: data-parallel over rows of input1 (1024 rows per core, 8 cores);
input2 and weights replicated.

Key algebraic simplifications used on-device:
  - softmax normalization (and max-subtraction) is skipped: LayerNorm is
    invariant to a positive per-row scale, so exp(s) @ v is normalized for
    free by LN (eps term differs by ~1e-3 relative — validated vs reference).
  - gamma is folded into Wo (diag(gamma) @ Wo) and beta/bo folded into an
    extra contraction row via an augmented ones-row, both on the host.

Dataflow per core:
  - in2 chunks [128,128] are PE-transposed (fp32); kT = Wk.T @ in2T computed
    with 4x column-tiled fp32 matmuls so kT chunk c lands at PSUM partitions
    32*(c%4) (the "banded" kstack layout); v = in2T.T @ Wv natural [n, 32].
  - qT is computed replicated into all 4 partition bands (column tiling).
  - scoresT[n, m] chunks via 4x row-tiled float32r matmuls (K=32): float32r
    streams at 1 cycle/column (fp32 is 4x slower); column tiling is
    incompatible with float32r on this compiler, row tiling is fine. All
    float32r matmul inputs must be produced by a rounding instruction
    (DVE/ACT output into a float32r tile), not DMA.
  - exp runs on ACT straight out of PSUM (the bottleneck engine, ~64us/core:
    8.4M elements at 1 elem/lane/cycle @ 1.2 GHz).
  - AV accumulates fusedT = v.T @ P.T in bf16 (column-tiled, 4 partition
    bands in one PSUM bank, summed at the end on DVE). bf16 error averages
    out over the ~1000s of softmax terms (<0.1% contribution).
  - LayerNorm stats via bn_stats/bn_aggr; rstd = exp(-0.5*ln(var+eps)) so
    the ACT table set stays in the ln/exp family (no table switch).
"""

import os
import sys

import numpy as np

N1 = 8192
N2 = 8192
DIN = 128
D = 32
DOUT = 128
NCORES = 8
MSH = N1 // NCORES          # rows per core
NCH = N2 // 128             # 64 in2 chunks
NG = NCH // 4               # 16 groups of 4 chunks
NMB = MSH // 512            # 2 m-passes of 512 rows
LN_EPS = 1e-5

_CACHE = {}


def _import_concourse():
    try:
        import concourse.bass  # noqa: F401
    except ImportError:
        for p in ("/opt/trn_rl_repo", os.path.expanduser("~/.axon_site/_ro/trn_rl_repo")):
            if os.path.isdir(p) and p not in sys.path:
                sys.path.insert(0, p)


def build(reps=1):
    """Build (and cache) the compiled single-core SPMD Bass program.

    reps > 1 repeats the whole computation serially (for wall-clock slope
    timing); the output is rewritten identically each rep.
    """
    key = ("nc", reps)
    if key in _CACHE:
        return _CACHE[key]
    _import_concourse()
    import concourse.bacc as bacc
    import concourse.tile as tile
    from concourse import mybir

    f32 = mybir.dt.float32
    AF = mybir.ActivationFunctionType
    OP = mybir.AluOpType

    f32r = mybir.dt.float32r
    bf16 = mybir.dt.bfloat16

    nc = bacc.Bacc(None, target_bir_lowering=False, debug=False)

    x1 = nc.dram_tensor("x1", [MSH, DIN], f32, kind="ExternalInput")
    x2 = nc.dram_tensor("x2", [N2, DIN], f32, kind="ExternalInput")
    wq_d = nc.dram_tensor("wq", [DIN, D], f32, kind="ExternalInput")
    wk_d = nc.dram_tensor("wk", [DIN, D], f32, kind="ExternalInput")
    wv_d = nc.dram_tensor("wv", [DIN, D], f32, kind="ExternalInput")
    bq4_d = nc.dram_tensor("bq4", [128, 1], f32, kind="ExternalInput")
    bk4_d = nc.dram_tensor("bk4", [128, 1], f32, kind="ExternalInput")
    bvb_d = nc.dram_tensor("bvb", [128, D], f32, kind="ExternalInput")
    woa_d = nc.dram_tensor("woa", [D + 1, DOUT], f32, kind="ExternalInput")
    id_d = nc.dram_tensor("ident", [128, 128], f32, kind="ExternalInput")
    eps_d = nc.dram_tensor("epsc", [128, 1], f32, kind="ExternalInput")
    out_d = nc.dram_tensor("out", [MSH, DOUT], f32, kind="ExternalOutput")

    from contextlib import ExitStack

    with tile.TileContext(nc) as tc, ExitStack() as outer:
        consts = outer.enter_context(tc.tile_pool(name="consts", bufs=1))
        persist = outer.enter_context(tc.tile_pool(name="persist", bufs=1))

        ident = consts.tile([128, 128], f32)
        nc.sync.dma_start(out=ident, in_=id_d[:])
        wq = consts.tile([DIN, D], f32)
        nc.sync.dma_start(out=wq, in_=wq_d[:])
        wk = consts.tile([DIN, D], f32)
        nc.sync.dma_start(out=wk, in_=wk_d[:])
        wv = consts.tile([DIN, D], f32)
        nc.sync.dma_start(out=wv, in_=wv_d[:])
        bq4 = consts.tile([128, 1], f32)
        nc.sync.dma_start(out=bq4, in_=bq4_d[:])
        bk4 = consts.tile([128, 1], f32)
        nc.sync.dma_start(out=bk4, in_=bk4_d[:])
        bvb = consts.tile([128, D], f32)
        nc.sync.dma_start(out=bvb, in_=bvb_d[:])
        woa = consts.tile([D + 1, DOUT], f32)
        nc.sync.dma_start(out=woa, in_=woa_d[:])
        epsc = consts.tile([128, 1], f32)
        nc.sync.dma_start(out=epsc, in_=eps_d[:])
        wv_r = consts.tile([DIN, D], f32r)
        nc.vector.tensor_copy(wv_r, wv)
        woa_r = consts.tile([D + 1, DOUT], f32r)
        nc.vector.tensor_copy(woa_r, woa)

        # Pull the exp table load (~2.7us) into the initial DMA window.
        warm = consts.tile([1, 8], f32)
        nc.scalar.activation(warm, ident[0:1, 0:8], AF.Exp)

        kstack = persist.tile([128, NG * 128], f32r)    # kT chunk c: [32*(c%4):+32, 128*(c//4):+128]
        vstack = persist.tile([128, NCH * D], bf16)     # v chunk c: [:, 32*c:+32]
        qt_rep = persist.tile([128, MSH], f32r)         # qT replicated in 4 bands
        x1t_all = persist.tile([128, MSH], f32)        # input1 shard transposed
        fusedT = persist.tile([128, MSH], f32)         # rows 0:32 = v.T @ P.T, rows 32:128 = 0
        for p0 in range(D, 128, 32):
            nc.vector.memset(fusedT[p0:p0 + 32, :], 0.0)

        for _rep in range(reps):
          with (
            tc.tile_pool(name="qload", bufs=1) as qload,
            tc.tile_pool(name="x2load", bufs=3) as x2load,
            tc.tile_pool(name="i2t_sb", bufs=6) as i2t_sbp,
            tc.tile_pool(name="pp_ps", bufs=2, space="PSUM") as pp_ps,
            tc.tile_pool(name="sc_ps", bufs=2, space="PSUM") as sc_ps,
            tc.tile_pool(name="av_ps", bufs=2, space="PSUM") as av_ps,
            tc.tile_pool(name="pt", bufs=4) as ptp,
            tc.tile_pool(name="tmp32", bufs=2) as tmp32p,
        ):
            # ---- q prep: x1 -> x1T -> qT replicated into 4 bands (+bq) ----
            x1_sb = qload.tile([128, MSH // 128, 128], f32)
            nc.sync.dma_start(
                out=x1_sb, in_=x1[:].rearrange("(t p) d -> p t d", p=128)
            )
            for t in range(MSH // 128):
                tps = sc_ps.tile([128, 128], f32, tag="sc")
                nc.tensor.transpose(tps, x1_sb[:, t, :], ident)
                nc.vector.tensor_copy(x1t_all[:, t * 128:(t + 1) * 128], tps)
            for t2 in range(MSH // 256):
                qps = sc_ps.tile([128, 256], f32, tag="sc")
                for j in range(4):
                    nc.tensor.matmul(
                        qps[32 * j:32 * (j + 1), :],
                        lhsT=wq,
                        rhs=x1t_all[:, t2 * 256:(t2 + 1) * 256],
                        start=True,
                        stop=True,
                        tile_position=(0, 32 * j),
                    )
                nc.vector.tensor_scalar_add(
                    qt_rep[:, t2 * 256:(t2 + 1) * 256], qps, bq4
                )

            # ---- streaming: in2 prep (group g) + attention pass 0 (group g) ----
            av_acc = [None, None]

            def prep_group(g):
                x2_sb = x2load.tile([128, 4, 128], f32, tag="x2")
                nc.sync.dma_start(
                    out=x2_sb,
                    in_=x2[g * 512:(g + 1) * 512, :].rearrange(
                        "(p t) d -> p t d", p=128
                    ),
                )
                i2t = []
                for j in range(4):
                    tps = pp_ps.tile([128, 128], f32, tag="pp")
                    nc.tensor.transpose(tps, x2_sb[:, j, :], ident)
                    sb = i2t_sbp.tile([128, 128], f32r, tag="i2t")
                    nc.vector.tensor_copy(sb, tps)
                    i2t.append(sb)
                for j in range(4):
                    c = 4 * g + j
                    vps = pp_ps.tile([128, D], f32, tag="pp")
                    nc.tensor.matmul(vps, lhsT=i2t[j], rhs=wv_r, start=True, stop=True)
                    nc.vector.tensor_add(vstack[:, D * c:D * (c + 1)], vps, bvb)
                kps = pp_ps.tile([128, 128], f32, tag="pp")
                for j in range(4):
                    nc.tensor.matmul(
                        kps[32 * j:32 * (j + 1), :],
                        lhsT=wk,
                        rhs=i2t[j].bitcast(f32),
                        start=True,
                        stop=True,
                        tile_position=(0, 32 * j),
                    )
                nc.vector.tensor_scalar_add(
                    kstack[:, g * 128:(g + 1) * 128], kps, bk4
                )

            def attn_group(p, g):
                m0 = p * 512
                for h in range(2):
                    sps = sc_ps.tile([128, 1024], f32, tag="sc")
                    for ci in range(2):
                        c = 4 * g + 2 * h + ci
                        j = c % 4
                        nc.tensor.matmul(
                            sps[:, 512 * ci:512 * (ci + 1)],
                            lhsT=kstack[32 * j:32 * (j + 1), g * 128:(g + 1) * 128],
                            rhs=qt_rep[32 * j:32 * (j + 1), m0:m0 + 512],
                            start=True,
                            stop=True,
                            tile_position=(32 * j, 0),
                        )
                    pt = ptp.tile([128, 1024], bf16, tag="pt")
                    nc.scalar.activation(pt, sps, AF.Exp)
                    for ci in range(2):
                        c = 4 * g + 2 * h + ci
                        j = c % 4
                        nc.tensor.matmul(
                            av_acc[p][32 * j:32 * (j + 1), :],
                            lhsT=vstack[:, D * c:D * (c + 1)],
                            rhs=pt[:, 512 * ci:512 * (ci + 1)],
                            start=(g == 0),
                            stop=(g == NG - 1),
                            tile_position=(0, 32 * j),
                            skip_group_check=True,
                        )

            def band_reduce(p):
                t1 = tmp32p.tile([D, 512], f32, tag="t1")
                nc.vector.tensor_copy(t1, av_acc[p][0:32, :])
                t2 = tmp32p.tile([D, 512], f32, tag="t2")
                nc.vector.tensor_add(t2, t1, av_acc[p][32:64, :])
                nc.vector.tensor_add(t1, t2, av_acc[p][64:96, :])
                nc.vector.tensor_add(
                    fusedT[0:D, p * 512:(p + 1) * 512], t1, av_acc[p][96:128, :]
                )

            av_acc[0] = av_ps.tile([128, 512], f32, tag="av", name="av0")
            av_acc[1] = av_ps.tile([128, 512], f32, tag="av", name="av1")
            for g in range(NG):
                prep_group(g)
                attn_group(0, g)
                attn_group(1, g)
            band_reduce(0)
            band_reduce(1)

          # ---- phase B: LayerNorm + output projection, batched over 8 blocks ----
          with (
            tc.tile_pool(name="pb_ps", bufs=2, space="PSUM") as pb_ps,
            tc.tile_pool(name="fb", bufs=2) as fbp,
            tc.tile_pool(name="cent", bufs=8) as centp,
            tc.tile_pool(name="stat", bufs=2) as statp,
            tc.tile_pool(name="lnagg", bufs=1) as lnagg,
            tc.tile_pool(name="naug", bufs=2) as naugp,
            tc.tile_pool(name="outsb", bufs=2) as outsbp,
        ):
            mv_all = lnagg.tile([128, MSH // 128, 2], f32)
            cents = []
            for b in range(MSH // 128):
                fps = pb_ps.tile([128, 128], f32, tag="pb")
                nc.tensor.transpose(fps, fusedT[:, b * 128:(b + 1) * 128], ident)
                fsb = fbp.tile([128, D], f32, tag="f")
                nc.vector.tensor_copy(fsb, fps[:, 0:D])
                st = statp.tile([128, 6], f32, tag="st")
                nc.vector.bn_stats(out=st, in_=fsb)
                nc.vector.bn_aggr(out=mv_all[:, b, :], in_=st)
                cent = centp.tile([128, 128], f32, tag="c")
                nc.vector.memset(cent[:, D:128], 0.0)
                nc.vector.tensor_scalar(
                    cent[:, 0:D], fsb, mv_all[:, b, 0:1], None, op0=OP.subtract
                )
                cents.append(cent)
            lnv = lnagg.tile([128, MSH // 128], f32)
            nc.scalar.activation(lnv, mv_all[:, :, 1], AF.Ln, bias=epsc)
            rstd = lnagg.tile([128, MSH // 128], f32)
            nc.scalar.activation(rstd, lnv, AF.Exp, scale=-0.5)
            for b in range(MSH // 128):
                nc.vector.tensor_scalar_mul(
                    cents[b][:, 0:D], cents[b][:, 0:D], rstd[:, b:b + 1]
                )
                nps = pb_ps.tile([128, 128], f32, tag="pb")
                nc.tensor.transpose(nps, cents[b], ident)
                na = naugp.tile([D + 1, 128], f32r, tag="na")
                nc.vector.tensor_copy(na[0:D, :], nps[0:D, :])
                nc.vector.tensor_scalar(
                    na[D:D + 1, :], ident[0:1, 0:128], 0.0, 1.0,
                    op0=OP.mult, op1=OP.add,
                )
                ops = pb_ps.tile([128, 128], f32, tag="pb")
                nc.tensor.matmul(ops, lhsT=na, rhs=woa_r, start=True, stop=True)
                osb = outsbp.tile([128, DOUT], f32, tag="o")
                nc.scalar.copy(osb, ops)
                nc.sync.dma_start(out=out_d[b * 128:(b + 1) * 128, :], in_=osb)

    nc.compile()
    _CACHE[key] = nc
    return nc


def host_inputs(input1, input2, Wq, bq, Wk, bk, Wv, bv, gamma, beta, Wo, bo):
    """Per-core input maps (host-side weight folding)."""
    f32 = np.float32
    input1 = np.ascontiguousarray(np.asarray(input1, f32))
    input2 = np.ascontiguousarray(np.asarray(input2, f32))
    woa = np.concatenate(
        [np.asarray(gamma, f32)[:, None] * np.asarray(Wo, f32),
         (np.asarray(beta, f32) @ np.asarray(Wo, f32) + np.asarray(bo, f32))[None, :]],
        axis=0,
    ).astype(f32)
    common = {
        "x2": input2,
        "wq": np.ascontiguousarray(np.asarray(Wq, f32)),
        "wk": np.ascontiguousarray(np.asarray(Wk, f32)),
        "wv": np.ascontiguousarray(np.asarray(Wv, f32)),
        "bq4": np.tile(np.asarray(bq, f32), 4)[:, None].copy(),
        "bk4": np.tile(np.asarray(bk, f32), 4)[:, None].copy(),
        "bvb": np.broadcast_to(np.asarray(bv, f32), (128, D)).copy(),
        "woa": woa,
        "ident": np.eye(128, dtype=f32),
        "epsc": np.full((128, 1), LN_EPS, f32),
    }
    return [
        dict(common, x1=input1[c * MSH:(c + 1) * MSH]) for c in range(NCORES)
    ]


def kernel(input1, input2, Wq, bq, Wk, bk, Wv, bv, gamma, beta, Wo, bo):
    _import_concourse()
    from concourse.bass_utils import run_bass_kernel_spmd

    nc = build()
    in_maps = host_inputs(
        input1, input2, Wq, bq, Wk, bk, Wv, bv, gamma, beta, Wo, bo
    )
    res = run_bass_kernel_spmd(nc, in_maps, list(range(NCORES)))
    return np.concatenate(
        [np.asarray(res.results[c]["out"]) for c in range(NCORES)], axis=0
    ).astype(np.float32)



# revision 1
# speedup vs baseline: 1.2634x; 1.2634x over previous
"""Trainium2 Bass kernel for fused attention + LayerNorm + projection.

Computation (per reference):
    q = input1 @ Wq + bq                       [8192, 32]
    k = input2 @ Wk + bk                       [8192, 32]
    v = input2 @ Wv + bv                       [8192, 32]
    P = softmax(q @ k.T, axis=-1)              [8192, 8192]
    fused = P @ v                              [8192, 32]
    out = LayerNorm(fused) * gamma + beta @ Wo + bo   [8192, 128]

Sharding: data-parallel over rows of input1 (1024 rows per core, 8 cores);
input2 and weights replicated.

Key algebraic simplifications used on-device:
  - softmax normalization (and max-subtraction) is skipped: LayerNorm is
    invariant to a positive per-row scale, so exp(s) @ v is normalized for
    free by LN (eps term differs by ~1e-3 relative — validated vs reference).
  - gamma is folded into Wo (diag(gamma) @ Wo) and beta/bo folded into an
    extra contraction row via an augmented ones-row, both on the host.

Dataflow per core:
  - in2 chunks [128,128] are PE-transposed (fp32); kT = Wk.T @ in2T computed
    with 4x column-tiled fp32 matmuls so kT chunk c lands at PSUM partitions
    32*(c%4) (the "banded" kstack layout); v = in2T.T @ Wv natural [n, 32].
  - qT is computed replicated into all 4 partition bands (column tiling).
  - scoresT[n, m] chunks via 4x row-tiled float32r matmuls (K=32): float32r
    streams at 1 cycle/column (fp32 is 4x slower); column tiling is
    incompatible with float32r on this compiler, row tiling is fine. All
    float32r matmul inputs must be produced by a rounding instruction
    (DVE/ACT output into a float32r tile), not DMA.
  - exp runs on ACT straight out of PSUM (the bottleneck engine, ~64us/core:
    8.4M elements at 1 elem/lane/cycle @ 1.2 GHz).
  - AV accumulates fusedT = v.T @ P.T in bf16 (column-tiled, 4 partition
    bands in one PSUM bank, summed at the end on DVE). bf16 error averages
    out over the ~1000s of softmax terms (<0.1% contribution).
  - LayerNorm stats via bn_stats/bn_aggr; rstd = exp(-0.5*ln(var+eps)) so
    the ACT table set stays in the ln/exp family (no table switch).
"""

import os
import sys

import numpy as np

N1 = 8192
N2 = 8192
DIN = 128
D = 32
DOUT = 128
NCORES = 8
MSH = N1 // NCORES          # rows per core
NCH = N2 // 128             # 64 in2 chunks
NG = NCH // 4               # 16 groups of 4 chunks
NMB = MSH // 512            # 2 m-passes of 512 rows
LN_EPS = 1e-5

_CACHE = {}


def _import_concourse():
    try:
        import concourse.bass  # noqa: F401
    except ImportError:
        for p in ("/opt/trn_rl_repo", os.path.expanduser("~/.axon_site/_ro/trn_rl_repo")):
            if os.path.isdir(p) and p not in sys.path:
                sys.path.insert(0, p)


def build(reps=1):
    """Build (and cache) the compiled single-core SPMD Bass program.

    reps > 1 repeats the whole computation serially (for wall-clock slope
    timing); the output is rewritten identically each rep.
    """
    key = ("nc", reps)
    if key in _CACHE:
        return _CACHE[key]
    _import_concourse()
    import concourse.bacc as bacc
    import concourse.tile as tile
    from concourse import mybir

    f32 = mybir.dt.float32
    AF = mybir.ActivationFunctionType
    OP = mybir.AluOpType

    f32r = mybir.dt.float32r
    bf16 = mybir.dt.bfloat16

    nc = bacc.Bacc(None, target_bir_lowering=False, debug=False)

    x1 = nc.dram_tensor("x1", [MSH, DIN], f32, kind="ExternalInput")
    x2 = nc.dram_tensor("x2", [N2, DIN], f32, kind="ExternalInput")
    wq_d = nc.dram_tensor("wq", [DIN, D], f32, kind="ExternalInput")
    wk_d = nc.dram_tensor("wk", [DIN, D], f32, kind="ExternalInput")
    wv_d = nc.dram_tensor("wv", [DIN, D], f32, kind="ExternalInput")
    bq4_d = nc.dram_tensor("bq4", [128, 1], f32, kind="ExternalInput")
    bk4_d = nc.dram_tensor("bk4", [128, 1], f32, kind="ExternalInput")
    bvb_d = nc.dram_tensor("bvb", [128, D], f32, kind="ExternalInput")
    woa_d = nc.dram_tensor("woa", [D + 1, DOUT], f32, kind="ExternalInput")
    id_d = nc.dram_tensor("ident", [128, 128], f32, kind="ExternalInput")
    eps_d = nc.dram_tensor("epsc", [128, 1], f32, kind="ExternalInput")
    out_d = nc.dram_tensor("out", [MSH, DOUT], f32, kind="ExternalOutput")

    from contextlib import ExitStack

    with tile.TileContext(nc) as tc, ExitStack() as outer:
        consts = outer.enter_context(tc.tile_pool(name="consts", bufs=1))
        persist = outer.enter_context(tc.tile_pool(name="persist", bufs=1))

        ident = consts.tile([128, 128], f32)
        nc.sync.dma_start(out=ident, in_=id_d[:])
        wq = consts.tile([DIN, D], f32)
        nc.sync.dma_start(out=wq, in_=wq_d[:])
        wk = consts.tile([DIN, D], f32)
        nc.sync.dma_start(out=wk, in_=wk_d[:])
        wv = consts.tile([DIN, D], f32)
        nc.sync.dma_start(out=wv, in_=wv_d[:])
        bq4 = consts.tile([128, 1], f32)
        nc.sync.dma_start(out=bq4, in_=bq4_d[:])
        bk4 = consts.tile([128, 1], f32)
        nc.sync.dma_start(out=bk4, in_=bk4_d[:])
        bvb = consts.tile([128, D], f32)
        nc.sync.dma_start(out=bvb, in_=bvb_d[:])
        woa = consts.tile([D + 1, DOUT], f32)
        nc.sync.dma_start(out=woa, in_=woa_d[:])
        epsc = consts.tile([128, 1], f32)
        nc.sync.dma_start(out=epsc, in_=eps_d[:])
        wv_r = consts.tile([DIN, D], f32r)
        nc.vector.tensor_copy(wv_r, wv)
        woa_r = consts.tile([D + 1, DOUT], f32r)
        nc.vector.tensor_copy(woa_r, woa)

        # Pull the exp table load (~2.7us) into the initial DMA window.
        warm = consts.tile([1, 8], f32)
        nc.scalar.activation(warm, ident[0:1, 0:8], AF.Exp)

        kstack = persist.tile([128, NG * 128], f32r)    # kT chunk c: [32*(c%4):+32, 128*(c//4):+128]
        vstack = persist.tile([128, NCH * D], bf16)     # v chunk c: [:, 32*c:+32]
        qt_rep = persist.tile([128, MSH], f32r)         # qT replicated in 4 bands
        x1t_all = persist.tile([128, MSH], f32)        # input1 shard transposed
        fusedT = persist.tile([128, MSH], f32)         # rows 0:32 = v.T @ P.T, rows 32:128 = 0
        for p0 in range(D, 128, 32):
            nc.vector.memset(fusedT[p0:p0 + 32, :], 0.0)

        for _rep in range(reps):
          with (
            tc.tile_pool(name="qload", bufs=1) as qload,
            tc.tile_pool(name="x2load", bufs=3) as x2load,
            tc.tile_pool(name="i2t_sb", bufs=6) as i2t_sbp,
            tc.tile_pool(name="pp_ps", bufs=2, space="PSUM") as pp_ps,
            tc.tile_pool(name="sc_ps", bufs=2, space="PSUM") as sc_ps,
            tc.tile_pool(name="av_ps", bufs=2, space="PSUM") as av_ps,
            tc.tile_pool(name="pt", bufs=4) as ptp,
            tc.tile_pool(name="tmp32", bufs=2) as tmp32p,
        ):
            # ---- q prep: x1 -> x1T -> qT replicated into 4 bands (+bq) ----
            x1_sb = qload.tile([128, MSH // 128, 128], f32)
            nc.sync.dma_start(
                out=x1_sb, in_=x1[:].rearrange("(t p) d -> p t d", p=128)
            )
            for t in range(MSH // 128):
                tps = sc_ps.tile([128, 128], f32, tag="sc")
                nc.tensor.transpose(tps, x1_sb[:, t, :], ident)
                nc.vector.tensor_copy(x1t_all[:, t * 128:(t + 1) * 128], tps)
            for t2 in range(MSH // 256):
                qps = sc_ps.tile([128, 256], f32, tag="sc")
                for j in range(4):
                    nc.tensor.matmul(
                        qps[32 * j:32 * (j + 1), :],
                        lhsT=wq,
                        rhs=x1t_all[:, t2 * 256:(t2 + 1) * 256],
                        start=True,
                        stop=True,
                        tile_position=(0, 32 * j),
                    )
                nc.vector.tensor_scalar_add(
                    qt_rep[:, t2 * 256:(t2 + 1) * 256], qps, bq4
                )

            # ---- streaming: in2 prep (group g) + attention pass 0 (group g) ----
            av_acc = [None, None]

            def prep_group(g):
                x2_sb = x2load.tile([128, 4, 128], f32, tag="x2")
                nc.sync.dma_start(
                    out=x2_sb,
                    in_=x2[g * 512:(g + 1) * 512, :].rearrange(
                        "(p t) d -> p t d", p=128
                    ),
                )
                i2t = []
                for j in range(4):
                    tps = pp_ps.tile([128, 128], f32, tag="pp")
                    nc.tensor.transpose(tps, x2_sb[:, j, :], ident)
                    sb = i2t_sbp.tile([128, 128], f32r, tag="i2t")
                    nc.vector.tensor_copy(sb, tps)
                    i2t.append(sb)
                for j in range(4):
                    c = 4 * g + j
                    vps = pp_ps.tile([128, D], f32, tag="pp")
                    nc.tensor.matmul(vps, lhsT=i2t[j], rhs=wv_r, start=True, stop=True)
                    nc.vector.tensor_add(vstack[:, D * c:D * (c + 1)], vps, bvb)
                kps = pp_ps.tile([128, 128], f32, tag="pp")
                for j in range(4):
                    nc.tensor.matmul(
                        kps[32 * j:32 * (j + 1), :],
                        lhsT=wk,
                        rhs=i2t[j].bitcast(f32),
                        start=True,
                        stop=True,
                        tile_position=(0, 32 * j),
                    )
                nc.vector.tensor_scalar_add(
                    kstack[:, g * 128:(g + 1) * 128], kps, bk4
                )

            def attn_group(p, g):
                m0 = p * 512
                for h in range(2):
                    sps = sc_ps.tile([128, 1024], f32, tag="sc")
                    for ci in range(2):
                        c = 4 * g + 2 * h + ci
                        j = c % 4
                        nc.tensor.matmul(
                            sps[:, 512 * ci:512 * (ci + 1)],
                            lhsT=kstack[32 * j:32 * (j + 1), g * 128:(g + 1) * 128],
                            rhs=qt_rep[32 * j:32 * (j + 1), m0:m0 + 512],
                            start=True,
                            stop=True,
                            tile_position=(32 * j, 0),
                        )
                    pt = ptp.tile([128, 1024], bf16, tag="pt")
                    nc.scalar.activation(pt, sps, AF.Exp)
                    for ci in range(2):
                        c = 4 * g + 2 * h + ci
                        j = c % 4
                        nc.tensor.matmul(
                            av_acc[p][32 * j:32 * (j + 1), :],
                            lhsT=vstack[:, D * c:D * (c + 1)],
                            rhs=pt[:, 512 * ci:512 * (ci + 1)],
                            start=(g == 0),
                            stop=(g == NG - 1),
                            tile_position=(0, 32 * j),
                            skip_group_check=True,
                        )

            def band_reduce(p):
                t1 = tmp32p.tile([D, 512], f32, tag="t1")
                nc.vector.tensor_copy(t1, av_acc[p][0:32, :])
                t2 = tmp32p.tile([D, 512], f32, tag="t2")
                nc.vector.tensor_add(t2, t1, av_acc[p][32:64, :])
                nc.vector.tensor_add(t1, t2, av_acc[p][64:96, :])
                nc.vector.tensor_add(
                    fusedT[0:D, p * 512:(p + 1) * 512], t1, av_acc[p][96:128, :]
                )

            av_acc[0] = av_ps.tile([128, 512], f32, tag="av", name="av0")
            av_acc[1] = av_ps.tile([128, 512], f32, tag="av", name="av1")
            for g in range(NG):
                prep_group(g)
                attn_group(0, g)
                attn_group(1, g)
            band_reduce(0)
            band_reduce(1)

          # ---- phase B: LayerNorm + output projection, batched over 8 blocks ----
          with (
            tc.tile_pool(name="pb_ps", bufs=2, space="PSUM") as pb_ps,
            tc.tile_pool(name="fb", bufs=2) as fbp,
            tc.tile_pool(name="cent", bufs=8) as centp,
            tc.tile_pool(name="stat", bufs=2) as statp,
            tc.tile_pool(name="lnagg", bufs=1) as lnagg,
            tc.tile_pool(name="naug", bufs=2) as naugp,
            tc.tile_pool(name="outsb", bufs=2) as outsbp,
        ):
            mv_all = lnagg.tile([128, MSH // 128, 2], f32)
            cents = []
            for b in range(MSH // 128):
                fps = pb_ps.tile([128, 128], f32, tag="pb")
                nc.tensor.transpose(fps, fusedT[:, b * 128:(b + 1) * 128], ident)
                fsb = fbp.tile([128, D], f32, tag="f")
                nc.vector.tensor_copy(fsb, fps[:, 0:D])
                st = statp.tile([128, 6], f32, tag="st")
                nc.vector.bn_stats(out=st, in_=fsb)
                nc.vector.bn_aggr(out=mv_all[:, b, :], in_=st)
                cent = centp.tile([128, 128], f32, tag="c")
                nc.vector.memset(cent[:, D:128], 0.0)
                nc.vector.tensor_scalar(
                    cent[:, 0:D], fsb, mv_all[:, b, 0:1], None, op0=OP.subtract
                )
                cents.append(cent)
            lnv = lnagg.tile([128, MSH // 128], f32)
            nc.scalar.activation(lnv, mv_all[:, :, 1], AF.Ln, bias=epsc)
            rstd = lnagg.tile([128, MSH // 128], f32)
            nc.scalar.activation(rstd, lnv, AF.Exp, scale=-0.5)
            for b in range(MSH // 128):
                nc.vector.tensor_scalar_mul(
                    cents[b][:, 0:D], cents[b][:, 0:D], rstd[:, b:b + 1]
                )
                nps = pb_ps.tile([128, 128], f32, tag="pb")
                nc.tensor.transpose(nps, cents[b], ident)
                na = naugp.tile([D + 1, 128], f32r, tag="na")
                nc.vector.tensor_copy(na[0:D, :], nps[0:D, :])
                nc.vector.tensor_scalar(
                    na[D:D + 1, :], ident[0:1, 0:128], 0.0, 1.0,
                    op0=OP.mult, op1=OP.add,
                )
                ops = pb_ps.tile([128, 128], f32, tag="pb")
                nc.tensor.matmul(ops, lhsT=na, rhs=woa_r, start=True, stop=True)
                osb = outsbp.tile([128, DOUT], f32, tag="o")
                nc.scalar.copy(osb, ops)
                nc.sync.dma_start(out=out_d[b * 128:(b + 1) * 128, :], in_=osb)

    nc.compile()
    _CACHE[key] = nc
    return nc


def host_inputs(input1, input2, Wq, bq, Wk, bk, Wv, bv, gamma, beta, Wo, bo):
    """Per-core input maps (host-side weight folding)."""
    f32 = np.float32
    input1 = np.ascontiguousarray(np.asarray(input1, f32))
    input2 = np.ascontiguousarray(np.asarray(input2, f32))
    woa = np.concatenate(
        [np.asarray(gamma, f32)[:, None] * np.asarray(Wo, f32),
         (np.asarray(beta, f32) @ np.asarray(Wo, f32) + np.asarray(bo, f32))[None, :]],
        axis=0,
    ).astype(f32)
    common = {
        "x2": input2,
        "wq": np.ascontiguousarray(np.asarray(Wq, f32)),
        "wk": np.ascontiguousarray(np.asarray(Wk, f32)),
        "wv": np.ascontiguousarray(np.asarray(Wv, f32)),
        "bq4": np.tile(np.asarray(bq, f32), 4)[:, None].copy(),
        "bk4": np.tile(np.asarray(bk, f32), 4)[:, None].copy(),
        "bvb": np.broadcast_to(np.asarray(bv, f32), (128, D)).copy(),
        "woa": woa,
        "ident": np.eye(128, dtype=f32),
        "epsc": np.full((128, 1), LN_EPS, f32),
    }
    return [
        dict(common, x1=input1[c * MSH:(c + 1) * MSH]) for c in range(NCORES)
    ]


def kernel(input1, input2, Wq, bq, Wk, bk, Wv, bv, gamma, beta, Wo, bo):
    _import_concourse()
    from concourse.bass_utils import run_bass_kernel_spmd

    nc = build()
    in_maps = host_inputs(
        input1, input2, Wq, bq, Wk, bk, Wv, bv, gamma, beta, Wo, bo
    )
    res = run_bass_kernel_spmd(nc, in_maps, list(range(NCORES)))
    return np.concatenate(
        [np.asarray(res.results[c]["out"]) for c in range(NCORES)], axis=0
    ).astype(np.float32)

